# revision 9
# baseline (speedup 1.0000x reference)
"""Trainium2 Bass kernel for DigitCapsuleLayer (single routing iteration).

Math: with num_iterations == 1 the routing coefficients are uniform 1/R, so

    v[b,c,o] = squash( (1/R) * sum_{r,i} x[b,r,i] * W[0,r,c,o,i] )

i.e. one big [B=128, K=32768] x [K=32768, N=1024] fp32 matmul followed by a
tiny squash nonlinearity.  W is 128 MB and read exactly once -> the kernel is
HBM-bound at ~144 MB of total traffic.

Sharding (8 cores): split the contraction dim K = (routes x incap) so each
core reads a distinct 16 MB slice of W (and a 2 MB slice of x) and computes a
[128, 1024] partial product.  The cross-core sum is done with AllToAll
collectives (each core collects the 8 partials for its 16-row batch slice and
sums them locally on the vector engine) -- much cheaper than ReduceScatter on
this runtime.  The output N dim is processed in two halves so the first
AllToAll (and the collective entry/rank-skew cost) hides under the second
half's DMA + matmul stream.  Each core applies the squash on its batch slice
and the host concatenates the 8 slices (pure data movement).
"""

import numpy as np

import concourse.bacc as bacc
import concourse.bass as bass
import concourse.bass_utils as bass_utils
import concourse.mybir as mybir
import concourse.tile as tile

# Problem shape (hardcoded per the kernel contract).
B, R, C, I, O = 128, 2048, 32, 16, 32
NCORES = 8
RSH = R // NCORES          # 256 routes per core
KS = RSH * I               # 4096 contraction rows per core
KC = KS // 128             # 32 k-chunks of 128
N = C * O                  # 1024
NH = N // 2                # 512 columns per half
BS = B // NCORES           # 16 batch rows per core after the exchange

# PE fp32 runs at 4 cycles/row; float32r streams at 1 cycle/row for N>=256
# with ~1e-4-level relative error.  Accumulation stays in fp32 PSUM.
USE_F32R = True
# W k-chunk DMA group sizes per half (sums to KC); small first group so the
# PE starts as early as possible.
W_GROUPS = [2, 6, 8, 8, 8]


def _build_program():
    nc = bacc.Bacc(
        "TRN2", target_bir_lowering=False, debug=False, num_devices=NCORES
    )
    f32 = mybir.dt.float32
    mm_dt = mybir.dt.float32r if USE_F32R else mybir.dt.float32

    xT = nc.dram_tensor("xT", [128, KC * B], mm_dt, kind="ExternalInput").ap()
    Wt = nc.dram_tensor("Wt", [128, KC, N], mm_dt, kind="ExternalInput").ap()
    # Output stored half-major [h, b_local, f_half]; host reassembles.
    out = nc.dram_tensor("out", [2, BS, NH], f32, kind="ExternalOutput").ap()

    with tile.TileContext(nc) as tc:
        with (
            tc.tile_pool(name="xpool", bufs=1) as xpool,
            tc.tile_pool(name="wpool", bufs=1) as wpool,
            tc.tile_pool(name="spool", bufs=1) as spool,
            tc.tile_pool(name="qpool", bufs=1) as qpool,
            tc.tile_pool(name="psum", bufs=1, space="PSUM") as psum_pool,
            tc.tile_pool(name="dram", bufs=1, space="DRAM") as dram_pool,
        ):
            # x slice resident in SBUF: [p=k%128, (kc, b)] = 2 MB.  SWDGE so
            # it runs concurrently with the first W chunk on the sync ring.
            x_sb = xpool.tile([128, KC * B], mm_dt)
            nc.gpsimd.dma_start(x_sb[:], xT[:])

            cc_ins = []
            cc_outs = []
            for h in range(2):
                # This half's W columns, all 32 k-chunks: [128, KC, 512] 8 MB.
                w_sb = wpool.tile(
                    [128, KC, NH], mm_dt, name=f"w_sb{h}", tag=f"w{h}"
                )
                g0 = 0
                for gsz in W_GROUPS:
                    nc.sync.dma_start(
                        w_sb[:, g0 : g0 + gsz, :],
                        Wt[:, g0 : g0 + gsz, h * NH : (h + 1) * NH],
                    )
                    g0 += gsz

                ps = psum_pool.tile([128, NH], f32, name=f"ps{h}", tag=f"ps{h}")
                for kc in range(KC):
                    nc.tensor.matmul(
                        ps,
                        x_sb[:, kc * B : (kc + 1) * B],
                        w_sb[:, kc, :],
                        start=(kc == 0),
                        stop=(kc == KC - 1),
                    )

                # Scale partial by 1/R while copying PSUM -> SBUF (DVE).
                s_sb = spool.tile([128, NH], f32, name=f"s_sb{h}", tag=f"s{h}")
                nc.vector.tensor_scalar_mul(s_sb[:], ps[:], 1.0 / R)

                # Exchange partials: after AllToAll, partition rows
                # [16j, 16j+16) of cc_out hold core j's partial for THIS
                # core's batch slice.
                cc_in = dram_pool.tile([B, NH], f32, name=f"cc_in{h}")
                cc_out = dram_pool.tile([B, NH], f32, name=f"cc_out{h}")
                nc.sync.dma_start(cc_in[:], s_sb[:])
                nc.gpsimd.collective_compute(
                    "AllToAll",
                    mybir.AluOpType.bypass,
                    replica_groups=[list(range(NCORES))],
                    ins=[cc_in.opt()],
                    outs=[cc_out.opt()],
                )
                cc_ins.append(cc_in)
                cc_outs.append(cc_out)

            # Per half: sum the 8 partials and apply squash pieces.
            # Partition layout: p = (b_local, ch) with ch = 8 chunks of 64
            # columns; within a chunk f = (cl, o) with cl = c % 2.
            sq_halves = []
            sv_halves = []
            for h in range(2):
                # SBUF [p=(b,ch), j, fl=64]: per-(p,j) 256 B contiguous.
                s8 = qpool.tile([128, NCORES, 64], f32, name=f"s8_{h}", tag=f"s8_{h}")
                nc.sync.dma_start(
                    s8[:],
                    cc_outs[h].rearrange(
                        "(j b) (ch fl) -> (b ch) j fl", j=NCORES, ch=8, fl=64
                    ),
                )
                # Sum over j (stride-permuted read, j innermost).
                sv = qpool.tile([128, 64], f32, name=f"sv{h}", tag=f"sv{h}")
                nc.vector.reduce_sum(
                    sv[:],
                    s8[:].rearrange("p j fl -> p fl j"),
                    axis=mybir.AxisListType.X,
                )
                sv_halves.append(sv)
                # Sum of squares over o within each (cl) group: [128, 2].
                s2 = qpool.tile([128, 2, 32], f32, name=f"s2_{h}", tag=f"s2_{h}")
                nc.vector.tensor_mul(
                    out=s2[:],
                    in0=sv[:].rearrange("p (cl o) -> p cl o", o=32),
                    in1=sv[:].rearrange("p (cl o) -> p cl o", o=32),
                )
                sq = qpool.tile([128, 2], f32, name=f"sq{h}", tag=f"sq{h}")
                nc.vector.reduce_sum(sq[:], s2[:], axis=mybir.AxisListType.X)
                sq_halves.append(sq)

            for h in range(2):
                sq, sv = sq_halves[h], sv_halves[h]
                rt = qpool.tile([128, 2], f32, name=f"rt{h}", tag=f"rt{h}")
                nc.scalar.sqrt(rt[:], sq[:])
                den = qpool.tile([128, 2], f32, name=f"den{h}", tag=f"den{h}")
                nc.vector.tensor_scalar_add(den[:], sq[:], 1.0)
                rec = qpool.tile([128, 2], f32, name=f"rec{h}", tag=f"rec{h}")
                nc.vector.reciprocal(rec[:], den[:])
                fac = qpool.tile([128, 2], f32, name=f"fac{h}", tag=f"fac{h}")
                nc.vector.tensor_mul(out=fac[:], in0=rt[:], in1=rec[:])
                v = qpool.tile([128, 2, 32], f32, name=f"v{h}", tag=f"v{h}")
                nc.vector.tensor_tensor(
                    v[:],
                    sv[:].rearrange("p (cl o) -> p cl o", o=32),
                    fac[:, :, None].to_broadcast((128, 2, 32)),
                    mybir.AluOpType.mult,
                )
                nc.sync.dma_start(
                    out[h].rearrange("b (ch fl) -> (b ch) fl", ch=8),
                    v[:].rearrange("p cl o -> p (cl o)"),
                )

    nc.compile()
    return nc


def _shard_inputs(x: np.ndarray, W: np.ndarray):
    """Per-core input layouts (pure data movement on host).

    Contraction index within core m: k = kc*128 + p with p = (rp, i),
    rp in [0,8); global route r = m*256 + kc*8 + rp.
    """
    in_maps = []
    for m in range(NCORES):
        xm = x[:, m * RSH : (m + 1) * RSH, :]          # (b, rr, i)
        xm = xm.reshape(B, KC, 8, I)                   # (b, kc, rp, i)
        x_prep = np.ascontiguousarray(
            xm.transpose(2, 3, 1, 0)                   # (rp, i, kc, b)
        ).reshape(128, KC * B)

        Wm = W[0, m * RSH : (m + 1) * RSH]             # (rr, c, o, i)
        Wm = Wm.reshape(KC, 8, C, O, I)                # (kc, rp, c, o, i)
        w_prep = np.ascontiguousarray(
            Wm.transpose(1, 4, 0, 2, 3)                # (rp, i, kc, c, o)
        ).reshape(128, KC, N)

        in_maps.append({"xT": x_prep, "Wt": w_prep})
    return in_maps


_CACHED_NC = None


def _get_nc():
    global _CACHED_NC
    if _CACHED_NC is None:
        _CACHED_NC = _build_program()
    return _CACHED_NC


def kernel(x: np.ndarray, W: np.ndarray, _trace: bool = False):
    x = np.ascontiguousarray(np.asarray(x, dtype=np.float32))
    W = np.ascontiguousarray(np.asarray(W, dtype=np.float32))
    nc = _get_nc()
    in_maps = _shard_inputs(x, W)
    res = bass_utils.run_bass_kernel_spmd(
        nc, in_maps, core_ids=list(range(NCORES)), trace=_trace
    )
    slices = [
        res.results[m]["out"].transpose(1, 0, 2).reshape(BS, N)
        for m in range(NCORES)
    ]
    out = np.concatenate(slices, axis=0).reshape(B, C, O, 1)
    if _trace:
        return out, res
    return out


# revision 15
# speedup vs baseline: 1.0247x; 1.0247x over previous
"""Trainium2 Bass kernel for DigitCapsuleLayer (single routing iteration).

Math: with num_iterations == 1 the routing coefficients are uniform 1/R, so

    v[b,c,o] = squash( (1/R) * sum_{r,i} x[b,r,i] * W[0,r,c,o,i] )

i.e. one big [B=128, K=32768] x [K=32768, N=1024] fp32 matmul followed by a
tiny squash nonlinearity.  W is 128 MB and read exactly once -> the kernel is
HBM-bound at ~144 MB of total traffic.

Sharding (8 cores): split the contraction dim K = (routes x incap) so each
core reads a distinct 16 MB slice of W (and a 2 MB slice of x) and computes a
[128, 1024] partial product.  The cross-core sum is done with AllToAll
collectives (each core collects the 8 partials for its 16-row batch slice and
sums them locally on the vector engine) -- much cheaper than ReduceScatter on
this runtime.  The output N dim is processed in two halves so the first
AllToAll (and the collective entry/rank-skew cost) hides under the second
half's DMA + matmul stream.  Each core applies the squash on its batch slice
and the host concatenates the 8 slices (pure data movement).
"""

import numpy as np

import concourse.bacc as bacc
import concourse.bass as bass
import concourse.bass_utils as bass_utils
import concourse.mybir as mybir
import concourse.tile as tile

# Problem shape (hardcoded per the kernel contract).
B, R, C, I, O = 128, 2048, 32, 16, 32
NCORES = 8
RSH = R // NCORES          # 256 routes per core
KS = RSH * I               # 4096 contraction rows per core
KC = KS // 128             # 32 k-chunks of 128
N = C * O                  # 1024
NH = N // 2                # 512 columns per half
BS = B // NCORES           # 16 batch rows per core after the exchange

# PE fp32 runs at 4 cycles/row; float32r streams at 1 cycle/row for N>=256
# with ~1e-4-level relative error.  Accumulation stays in fp32 PSUM.
USE_F32R = True
# W k-chunk DMA group sizes per half (sums to KC); small first group so the
# PE starts as early as possible.
W_GROUPS = [2, 6, 8, 8, 8]


def _build_program():
    nc = bacc.Bacc(
        "TRN2", target_bir_lowering=False, debug=False, num_devices=NCORES
    )
    f32 = mybir.dt.float32
    mm_dt = mybir.dt.float32r if USE_F32R else mybir.dt.float32

    xT = nc.dram_tensor("xT", [128, KC * B], mm_dt, kind="ExternalInput").ap()
    # Half-major W so each half's stream is fully contiguous per partition.
    Wt = nc.dram_tensor("Wt", [2, 128, KC, NH], mm_dt, kind="ExternalInput").ap()
    # Output stored half-major [h, b_local, f_half]; host reassembles.
    out = nc.dram_tensor("out", [2, BS, NH], f32, kind="ExternalOutput").ap()

    with tile.TileContext(nc) as tc:
        with (
            tc.tile_pool(name="xpool", bufs=1) as xpool,
            tc.tile_pool(name="wpool", bufs=1) as wpool,
            tc.tile_pool(name="spool", bufs=1) as spool,
            tc.tile_pool(name="qpool", bufs=1) as qpool,
            tc.tile_pool(name="psum", bufs=1, space="PSUM") as psum_pool,
            tc.tile_pool(name="dram", bufs=1, space="DRAM") as dram_pool,
        ):
            # x slice resident in SBUF: [p=k%128, (kc, b)] = 2 MB.  SWDGE so
            # it runs concurrently with the first W chunk on the sync ring.
            x_sb = xpool.tile([128, KC * B], mm_dt)
            nc.gpsimd.dma_start(x_sb[:], xT[:])

            cc_ins = []
            cc_outs = []
            for h in range(2):
                # This half's W columns, all 32 k-chunks: [128, KC, 512] 8 MB.
                w_sb = wpool.tile(
                    [128, KC, NH], mm_dt, name=f"w_sb{h}", tag=f"w{h}"
                )
                # The sync ring carries ONLY the W stream (HWDGE rings are
                # FIFO per engine -- any dependent DMA here would stall it).
                g0 = 0
                for gsz in W_GROUPS:
                    nc.sync.dma_start(
                        w_sb[:, g0 : g0 + gsz, :],
                        Wt[h, :, g0 : g0 + gsz, :],
                    )
                    g0 += gsz

                ps = psum_pool.tile([128, NH], f32, name=f"ps{h}", tag=f"ps{h}")
                for kc in range(KC):
                    nc.tensor.matmul(
                        ps,
                        x_sb[:, kc * B : (kc + 1) * B],
                        w_sb[:, kc, :],
                        start=(kc == 0),
                        stop=(kc == KC - 1),
                    )

                # Scale partial by 1/R while copying PSUM -> SBUF (DVE).
                s_sb = spool.tile([128, NH], f32, name=f"s_sb{h}", tag=f"s{h}")
                nc.vector.tensor_scalar_mul(s_sb[:], ps[:], 1.0 / R)

                # Exchange partials: after AllToAll, partition rows
                # [16j, 16j+16) of cc_out hold core j's partial for THIS
                # core's batch slice.
                cc_in = dram_pool.tile([B, NH], f32, name=f"cc_in{h}")
                cc_out = dram_pool.tile([B, NH], f32, name=f"cc_out{h}")
                nc.scalar.dma_start(cc_in[:], s_sb[:])
                nc.gpsimd.collective_compute(
                    "AllToAll",
                    mybir.AluOpType.bypass,
                    replica_groups=[list(range(NCORES))],
                    ins=[cc_in.opt()],
                    outs=[cc_out.opt()],
                )
                cc_ins.append(cc_in)
                cc_outs.append(cc_out)

            # Per half: sum the 8 partials and apply squash pieces.
            # Partition layout: p = (b_local, ch) with ch = 8 chunks of 64
            # columns; within a chunk f = (cl, o) with cl = c % 2.
            sq_halves = []
            sv_halves = []
            for h in range(2):
                # SBUF [p=(b,ch), j, fl=64]: per-(p,j) 256 B contiguous.
                s8 = qpool.tile([128, NCORES, 64], f32, name=f"s8_{h}", tag=f"s8_{h}")
                nc.scalar.dma_start(
                    s8[:],
                    cc_outs[h].rearrange(
                        "(j b) (ch fl) -> (b ch) j fl", j=NCORES, ch=8, fl=64
                    ),
                )
                # Sum over j (stride-permuted read, j innermost).
                sv = qpool.tile([128, 64], f32, name=f"sv{h}", tag=f"sv{h}")
                nc.vector.reduce_sum(
                    sv[:],
                    s8[:].rearrange("p j fl -> p fl j"),
                    axis=mybir.AxisListType.X,
                )
                sv_halves.append(sv)
                # Sum of squares over o within each (cl) group: [128, 2].
                s2 = qpool.tile([128, 2, 32], f32, name=f"s2_{h}", tag=f"s2_{h}")
                nc.vector.tensor_mul(
                    out=s2[:],
                    in0=sv[:].rearrange("p (cl o) -> p cl o", o=32),
                    in1=sv[:].rearrange("p (cl o) -> p cl o", o=32),
                )
                sq = qpool.tile([128, 2], f32, name=f"sq{h}", tag=f"sq{h}")
                nc.vector.reduce_sum(sq[:], s2[:], axis=mybir.AxisListType.X)
                sq_halves.append(sq)

            for h in range(2):
                sq, sv = sq_halves[h], sv_halves[h]
                rt = qpool.tile([128, 2], f32, name=f"rt{h}", tag=f"rt{h}")
                nc.scalar.sqrt(rt[:], sq[:])
                den = qpool.tile([128, 2], f32, name=f"den{h}", tag=f"den{h}")
                nc.vector.tensor_scalar_add(den[:], sq[:], 1.0)
                rec = qpool.tile([128, 2], f32, name=f"rec{h}", tag=f"rec{h}")
                nc.vector.reciprocal(rec[:], den[:])
                fac = qpool.tile([128, 2], f32, name=f"fac{h}", tag=f"fac{h}")
                nc.vector.tensor_mul(out=fac[:], in0=rt[:], in1=rec[:])
                v = qpool.tile([128, 2, 32], f32, name=f"v{h}", tag=f"v{h}")
                nc.vector.tensor_tensor(
                    v[:],
                    sv[:].rearrange("p (cl o) -> p cl o", o=32),
                    fac[:, :, None].to_broadcast((128, 2, 32)),
                    mybir.AluOpType.mult,
                )
                nc.scalar.dma_start(
                    out[h].rearrange("b (ch fl) -> (b ch) fl", ch=8),
                    v[:].rearrange("p cl o -> p (cl o)"),
                )

    nc.compile()
    return nc


def _shard_inputs(x: np.ndarray, W: np.ndarray):
    """Per-core input layouts (pure data movement on host).

    Contraction index within core m: k = kc*128 + p with p = (rp, i),
    rp in [0,8); global route r = m*256 + kc*8 + rp.
    """
    in_maps = []
    for m in range(NCORES):
        xm = x[:, m * RSH : (m + 1) * RSH, :]          # (b, rr, i)
        xm = xm.reshape(B, KC, 8, I)                   # (b, kc, rp, i)
        x_prep = np.ascontiguousarray(
            xm.transpose(2, 3, 1, 0)                   # (rp, i, kc, b)
        ).reshape(128, KC * B)

        Wm = W[0, m * RSH : (m + 1) * RSH]             # (rr, c, o, i)
        Wm = Wm.reshape(KC, 8, 2, C // 2, O, I)        # (kc, rp, h, cl16, o, i)
        w_prep = np.ascontiguousarray(
            Wm.transpose(2, 1, 5, 0, 3, 4)             # (h, rp, i, kc, cl16, o)
        ).reshape(2, 128, KC, NH)

        in_maps.append({"xT": x_prep, "Wt": w_prep})
    return in_maps


_CACHED_NC = None


def _get_nc():
    global _CACHED_NC
    if _CACHED_NC is None:
        _CACHED_NC = _build_program()
    return _CACHED_NC


def kernel(x: np.ndarray, W: np.ndarray, _trace: bool = False):
    x = np.ascontiguousarray(np.asarray(x, dtype=np.float32))
    W = np.ascontiguousarray(np.asarray(W, dtype=np.float32))
    nc = _get_nc()
    in_maps = _shard_inputs(x, W)
    res = bass_utils.run_bass_kernel_spmd(
        nc, in_maps, core_ids=list(range(NCORES)), trace=_trace
    )
    slices = [
        res.results[m]["out"].transpose(1, 0, 2).reshape(BS, N)
        for m in range(NCORES)
    ]
    out = np.concatenate(slices, axis=0).reshape(B, C, O, 1)
    if _trace:
        return out, res
    return out
